# revision 50
# baseline (speedup 1.0000x reference)
"""GAT (2-layer, 8-head) Bass kernel for 8 Trainium2 NeuronCores.

Strategy (row-sharded attention):
  - Core d owns 512 rows (nodes) i in [512d, 512d+512).
  - Layer-1: each core computes h_k = x_d @ W_k for its rows (all 8 heads),
    plus score vectors; one AllGather shares [v_j*h_j | v_j | z_j] for all
    nodes; each core then computes its row-block of masked attention and
    h'_k via PE matmuls, using the separable-exponential decomposition
        exp(leakyrelu(s1_i + s2_j)) = u_i * v_j * max(w_i * z_j, 1)
    with u = exp(.2 s1) (cancels in softmax), v = exp(.2 s2),
    w = exp(.8 s1), z = exp(.8 s2).
    The unnormalized masked score matrix (transposed, [j,i]-layout) is
        U[j,i] = max(Wb[j,i] * z_j, 1) * mask[j,i]      (v folded into h)
    built with one dual-scalar TENSOR_SCALAR + one TENSOR_TENSOR per tile,
    then contracted on the PE against [v*h | v] to get numerator and
    denominator in one PSUM accumulation.
  - Layer-2: same pipeline once more on z = mean_k elu(h'_k).

The mask (adj > 0) is pre-transposed and bf16-encoded on the host; adj is
never touched on-device in int32 form.
"""

import numpy as np
import ml_dtypes

import concourse.bass as bass
import concourse.bacc as bacc
import concourse.tile as tile
import concourse.mybir as mybir
from concourse.bass_utils import run_bass_kernel_spmd
from concourse.masks import make_identity

dt = mybir.dt
Alu = mybir.AluOpType
Act = mybir.ActivationFunctionType
AX = mybir.AxisListType

NCORES = 8
N, F, NH, KH, NO = 4096, 512, 64, 8, 56
P = 128
R = N // NCORES          # rows per core = 512
IT = R // P              # i-tiles per core = 4
NB = N // P              # j-blocks = 32
GRP = 4                  # j-blocks per TT group
NG = NB // GRP           # groups = 8
bf16 = ml_dtypes.bfloat16

_CACHE: dict = {}


def _build():
    nc = bacc.Bacc("TRN2", target_bir_lowering=False, debug=False,
                   num_devices=NCORES)

    # ---- I/O -----------------------------------------------------------
    xT_d = nc.dram_tensor("xT", [P, 4, R], dt.bfloat16, kind="ExternalInput")
    maskT_d = nc.dram_tensor("maskT", [P, NB, R], dt.bfloat16,
                             kind="ExternalInput")
    wkt_d = nc.dram_tensor("wkt", [P, KH // 2, 4, P], dt.bfloat16,
                           kind="ExternalInput")
    waall_d = nc.dram_tensor("waall", [P, 4, 2 * KH], dt.bfloat16,
                             kind="ExternalInput")
    wab_d = nc.dram_tensor("wab", [P, KH // 2, 4, 33], dt.bfloat16,
                           kind="ExternalInput")
    wout_d = nc.dram_tensor("wout", [NH, NO], dt.bfloat16,
                            kind="ExternalInput")
    aout_d = nc.dram_tensor("aout", [NH, 2], dt.bfloat16,
                            kind="ExternalInput")
    sel_d = nc.dram_tensor("sel", [P, 4, NH + 1], dt.float32,
                           kind="ExternalInput")
    xTf_d = nc.dram_tensor("xTf", [P, 4, N], dt.bfloat16,
                           kind="ExternalInput")
    wk0a_d = nc.dram_tensor("wk0a", [P, 4, NH + 1], dt.bfloat16,
                            kind="ExternalInput")
    out_d = nc.dram_tensor("out", [R, NO], dt.float32, kind="ExternalOutput")

    with tile.TileContext(nc) as tc:
        _emit(nc, tc, xT_d, maskT_d, wkt_d, waall_d, wab_d, wout_d, aout_d,
              sel_d, xTf_d, wk0a_d, out_d)

    nc.compile()
    return nc


def _emit(nc, tc, xT_d, maskT_d, wkt_d, waall_d, wab_d, wout_d, aout_d,
              sel_d, xTf_d, wk0a_d, out_d):
    from contextlib import ExitStack
    ctx = ExitStack()
    with ctx:
        const = ctx.enter_context(tc.tile_pool(name="const", bufs=1))
        dram = ctx.enter_context(tc.tile_pool(name="dram", bufs=1,
                                              space="DRAM"))
        # PSUM pools: phase-A style (bufs=1) and attention (bufs=2)
        pa = ctx.enter_context(tc.tile_pool(name="pa", bufs=1, space="PSUM"))
        pb = ctx.enter_context(tc.tile_pool(name="pb", bufs=2, space="PSUM"))
        pt = ctx.enter_context(tc.tile_pool(name="pt", bufs=1, space="PSUM"))
        # SBUF working pools
        sp = ctx.enter_context(tc.tile_pool(name="sp", bufs=3))
        wp = ctx.enter_context(tc.tile_pool(name="wp", bufs=8))
        hp = ctx.enter_context(tc.tile_pool(name="hp", bufs=2))
        pp2 = ctx.enter_context(tc.tile_pool(name="pp2", bufs=4))
        tp = ctx.enter_context(tc.tile_pool(name="tp", bufs=3))
        up = ctx.enter_context(tc.tile_pool(name="up", bufs=3))
        bp = ctx.enter_context(tc.tile_pool(name="bp", bufs=3))
        cp = ctx.enter_context(tc.tile_pool(name="cp", bufs=2))
        zp = ctx.enter_context(tc.tile_pool(name="zp", bufs=1))

        # ---- resident loads (all partition-major, contiguous) ---------
        xT_sb = const.tile([P, 4, R], dt.bfloat16)
        nc.sync.dma_start(out=xT_sb, in_=xT_d.ap())
        wkp_sb = const.tile([P, KH // 2, 4, P], dt.bfloat16)
        nc.sync.dma_start(out=wkp_sb, in_=wkt_d.ap())
        waall_sb = const.tile([P, 4, 2 * KH], dt.bfloat16)
        nc.sync.dma_start(out=waall_sb, in_=waall_d.ap())
        wab_sb = const.tile([P, KH // 2, 4, 33], dt.bfloat16)
        nc.sync.dma_start(out=wab_sb, in_=wab_d.ap())
        wout_sb = const.tile([NH, NO], dt.bfloat16)
        nc.sync.dma_start(out=wout_sb, in_=wout_d.ap())
        sel_sb = const.tile([P, 4, NH + 1], dt.float32)
        nc.sync.dma_start(out=sel_sb, in_=sel_d.ap())
        aout_sb = const.tile([NH, 2], dt.bfloat16)
        nc.sync.dma_start(out=aout_sb, in_=aout_d.ap())
        xTf_sb = const.tile([P, 4, N], dt.bfloat16)
        nc.gpsimd.dma_start(out=xTf_sb, in_=xTf_d.ap())
        wk0a_sb = const.tile([P, 4, NH + 1], dt.bfloat16)
        nc.gpsimd.dma_start(out=wk0a_sb, in_=wk0a_d.ap())
        mask_sb = const.tile([P, NB, R], dt.bfloat16)
        for mh in range(2):
            lo, hi = mh * (NB // 2), (mh + 1) * (NB // 2)
            meng = nc.scalar if mh == 0 else nc.sync
            meng.dma_start(out=mask_sb[:, lo:hi, :],
                           in_=maskT_d.ap()[:, lo:hi, :])

        idb = const.tile([P, P], dt.bfloat16)
        make_identity(nc, idb)
        idf = const.tile([P, P], dt.float32)
        make_identity(nc, idf)
        neg1 = const.tile([P, 1], dt.float32)
        nc.vector.memset(neg1, -1.0)
        log8 = const.tile([P, 1], dt.float32)
        nc.vector.memset(log8, -2.0794415416798357)  # ln(1/8)
        ones1 = const.tile([1, P], dt.bfloat16)
        nc.vector.memset(ones1, 1.0)
        zero1 = const.tile([P, 1], dt.bfloat16)
        nc.vector.memset(zero1, 0.0)

        # persistent accumulators
        u_all = zp.tile([P, KH, IT, NH], dt.float32)   # h'_k per head (divided)
        zacc = zp.tile([P, IT, NH], dt.float32)        # mean elu
        zbf = zp.tile([P, IT, NH], dt.bfloat16)

        # head 0 is computed locally from the replicated full x during the
        # first-collective barrier window; only heads 1-7 are gathered.
        AG_SPLIT = [1, 2, 2, 2]                    # heads per gather wave
        ag_ins, ag_outs = [], []
        for nk in AG_SPLIT:
            ag_ins.append(dram.tile([R, nk, NH + 2], dt.bfloat16,
                                    name=f"agi{len(ag_ins)}"))
            ag_outs.append(dram.tile([N, nk, NH + 2], dt.bfloat16,
                                     addr_space="Shared",
                                     name=f"ago{len(ag_outs)}"))

        def head_wave(k):
            return (0, 0) if k == 1 else ((k - 2) // 2 + 1, (k - 2) % 2)
        ag2_in = dram.tile([R, NO + 2], dt.bfloat16)
        ag2_out = dram.tile([N, NO + 2], dt.bfloat16, addr_space="Shared")



        # ================= phase A: batched h + payload =================
        # h is computed directly in natural [i, hid] layout (x^T slices
        # stationary, W streamed), s-columns for all 8 heads come from one
        # host-precomputed (W @ a) stationary, and the w-rows from a tiny
        # dedicated matmul — no transposes, no cross-engine chains.
        ps_sall = pa.tile([P, IT, 2 * KH], dt.float32, tag="sall")
        scl_sb = sp.tile([P, IT, 2 * KH], dt.bfloat16, tag="scl", bufs=1)

        def hn_batch(half):
            prs = [2 * half, 2 * half + 1]
            ps_hns = {pr: pa.tile([P, IT, P], dt.float32, tag="hn", bufs=2,
                                  name=f"ps_hn{pr}")
                      for pr in prs}
            for isl in range(IT):
                sl = slice(isl * P, (isl + 1) * P)
                for fb in range(4):
                    lhsT = xT_sb[:, fb, sl]
                    for pr in prs:
                        nc.tensor.matmul(ps_hns[pr][:, isl, :], lhsT,
                                         wkp_sb[:, pr, fb, :],
                                         start=(fb == 0), stop=(fb == 3))
                    if half == 0:
                        nc.tensor.matmul(ps_sall[:, isl, :], lhsT,
                                         waall_sb[:, fb, :],
                                         start=(fb == 0), stop=(fb == 3))
                if half == 0:
                    nc.any.tensor_copy(scl_sb[:, isl, :], ps_sall[:, isl, :])
            return ps_hns

        def pair_payload(pr, ps_hn):
            Wbs = []
            ps_s = pa.tile([33, R], dt.float32, tag="srow")
            for fb in range(4):
                nc.tensor.matmul(ps_s, wab_sb[:, pr, fb, :],
                                 xT_sb[:, fb, :],
                                 start=(fb == 0), stop=(fb == 3))
            for hh in range(2):
                k = 2 * pr + hh
                wrow = sp.tile([1, R], dt.bfloat16, tag="wrow")
                nc.scalar.activation(wrow, ps_s[32 * hh:32 * hh + 1, :],
                                     Act.Exp, scale=0.8)
                ps_wb = pa.tile([P, R], dt.float32, tag="wb")
                nc.tensor.matmul(ps_wb, ones1, wrow, start=True, stop=True)
                Wb = wp.tile([P, R], dt.bfloat16, tag="Wb")
                nc.vector.tensor_copy(Wb, ps_wb)
                Wbs.append(Wb)
                if k == 0:
                    continue        # head 0's j-side is computed locally
                vcol = sp.tile([P, IT], dt.float32, tag="vcol")
                nc.scalar.activation(vcol, scl_sb[:, :, 2 * k + 1], Act.Exp,
                                     scale=0.2)
                pay = pp2.tile([P, IT, NH + 2], dt.bfloat16, tag="pay")
                for isl in range(IT):
                    nc.scalar.activation(
                        pay[:, isl, 0:NH],
                        ps_hn[:, isl, hh * NH:(hh + 1) * NH],
                        Act.Copy, scale=vcol[:, isl:isl + 1])
                nc.vector.tensor_copy(pay[:, :, NH], vcol)
                nc.scalar.activation(pay[:, :, NH + 1], scl_sb[:, :, 2 * k + 1],
                                     Act.Exp, scale=0.8)
                wave, kk = head_wave(k)
                nc.sync.dma_start(
                    out=ag_ins[wave][:, kk, :].rearrange(
                        "(isl p) c -> p isl c", p=P),
                    in_=pay)
            return Wbs

        Wb_k = []
        for half in range(2):
            ps_hns = hn_batch(half)
            for pr in (2 * half, 2 * half + 1):
                Wb_k += pair_payload(pr, ps_hns[pr])
                nc.gpsimd.collective_compute(
                    "AllGather", Alu.bypass,
                    ins=[ag_ins[pr].opt()], outs=[ag_outs[pr].opt()],
                    replica_groups=[list(range(NCORES))])

        # ---- local full-graph payload for head 0 -----------------------
        # Runs in the otherwise-idle window while the runtime's
        # first-collective barrier + wave-0 AllGather complete: h, v, z
        # for ALL 4096 nodes of head 0 from the replicated x^T.
        hsb0 = const.tile([P, NB, NH + 2], dt.bfloat16)
        for c in range(NG):
            ps_a0 = pt.tile([P, 4, NH + 1], dt.float32, tag="tr")
            for jj in range(4):
                jb = 4 * c + jj
                for fb in range(4):
                    nc.tensor.matmul(ps_a0[:, jj, :],
                                     xTf_sb[:, fb, jb * P:(jb + 1) * P],
                                     wk0a_sb[:, fb, :],
                                     start=(fb == 0), stop=(fb == 3))
            vq = sp.tile([P, 4], dt.float32, tag="vq")
            nc.scalar.activation(vq, ps_a0[:, :, NH], Act.Exp, scale=0.2)
            nc.scalar.activation(hsb0[:, 4 * c:4 * c + 4, NH + 1],
                                 ps_a0[:, :, NH], Act.Exp, scale=0.8)
            nc.vector.tensor_copy(hsb0[:, 4 * c:4 * c + 4, NH], vq)
            for jj in range(4):
                nc.vector.tensor_scalar(hsb0[:, 4 * c + jj, 0:NH],
                                        ps_a0[:, jj, 0:NH],
                                        vq[:, jj:jj + 1], None, Alu.mult)

        # ================= phase B: per-head attention ==================
        def attention(k, Wb, hsb_src, ncols, ps_tag, hsb_local=None):
            """hsb_src: dram AP [P, NB, ncols+1]-gatherable; returns psum_t
            [P, IT, ncols] (last col = denominator)."""
            if hsb_local is not None:
                hsb = hsb_local
            else:
                hsb = bp.tile([P, NB, ncols + 1], dt.bfloat16, tag="hsb")
                nc.sync.dma_start(out=hsb, in_=hsb_src)
            zf = bp.tile([P, NB], dt.float32, tag="zf")
            nc.vector.tensor_copy(zf, hsb[:, :, ncols])
            ps_nm = pb.tile([ncols, R], dt.float32, tag="nm")
            # Per group of 4 j-blocks: the first NACT tiles are produced on
            # the ACT engine as T = relu(z*Wb - 1); their missing "+1" is
            # recovered by streaming the resident mask tile itself through
            # the PE into the same PSUM accumulation (numer += sum_j m*h).
            # The other tiles use the DVE dual-scalar max(z*Wb, 1) form.
            # One plain TENSOR_TENSOR applies the mask to all 4 tiles.
            NACT = 2
            for g in range(NG):
                Tg = tp.tile([P, GRP, R], dt.bfloat16, tag="T")
                for q in range(NACT):
                    jb = g * GRP + q
                    nc.scalar.activation(Tg[:, q, :], Wb, Act.Relu,
                                         bias=neg1[:, 0:1],
                                         scale=zf[:, jb:jb + 1])
                for q in range(NACT, GRP):
                    jb = g * GRP + q
                    nc.vector.tensor_scalar(Tg[:, q, :], Wb,
                                            zf[:, jb:jb + 1], 1.0,
                                            Alu.mult, Alu.max)
                Ug = up.tile([P, GRP, R], dt.bfloat16, tag="U")
                nc.vector.tensor_tensor(Ug, Tg,
                                        mask_sb[:, g * GRP:(g + 1) * GRP, :],
                                        Alu.mult)
                for q in range(GRP):
                    jb = g * GRP + q
                    nc.tensor.matmul(ps_nm, hsb[:, jb, 0:ncols], Ug[:, q, :],
                                     start=(jb == 0), stop=(jb == NB - 1))
                    if q < NACT:
                        nc.tensor.matmul(ps_nm, hsb[:, jb, 0:ncols],
                                         mask_sb[:, jb, :],
                                         start=False, stop=False)
            nmf = bp.tile([ncols, R], dt.float32, tag="nmf")
            nc.any.tensor_copy(nmf, ps_nm)
            ps_t = pt.tile([P, IT, ncols], dt.float32, tag=ps_tag)
            for isl in range(IT):
                sl = slice(isl * P, (isl + 1) * P)
                nc.tensor.transpose(ps_t[:, isl, :], nmf[:, sl],
                                    idf[0:ncols, 0:ncols])
            return ps_t

        # phase B + incremental phase C: after each head's normalize, its
        # elu contribution is folded into zacc (init -1, += relu(u)/8 +
        # min(exp(u),1)/8 per head) so no serial elu/mean block remains
        # after the last head.
        nc.vector.memset(zacc, -1.0)
        for k in range(KH):
            if k == 0:
                ps_t = attention(0, Wb_k[0], None, NH + 1, "tr",
                                 hsb_local=hsb0)
            else:
                wave, kk = head_wave(k)
                src = ag_outs[wave][:, kk, :].rearrange(
                    "(jb p) c -> p jb c", p=P)
                ps_t = attention(k, Wb_k[k], src, NH + 1, "tr")
            for isl in range(IT):
                rc = sp.tile([P, 1], dt.float32, tag="rc")
                nc.vector.reciprocal(rc, ps_t[:, isl, NH:NH + 1])
                nc.scalar.activation(u_all[:, k, isl, :],
                                     ps_t[:, isl, 0:NH],
                                     Act.Copy, scale=rc[:, 0:1])
            uk = u_all[:, k, :, :]
            dp = cp.tile([P, IT, NH], dt.float32, tag="cdp")
            nc.scalar.activation(dp, uk, Act.Relu, scale=0.125)
            rn = cp.tile([P, IT, NH], dt.float32, tag="crn")
            nc.scalar.activation(rn, uk, Act.Relu, scale=-1.0)
            bq = cp.tile([P, IT, NH], dt.float32, tag="cbq")
            nc.scalar.activation(bq, rn, Act.Exp, bias=log8[:, 0:1],
                                 scale=-1.0)
            nc.gpsimd.tensor_tensor(zacc, zacc, dp, Alu.add)
            nc.gpsimd.tensor_tensor(zacc, zacc, bq, Alu.add)
        nc.vector.tensor_copy(zbf, zacc)

        # ================= phase D: layer-2 h2 + payload ================
        ps_zT = pa.tile([NH, R], dt.bfloat16, tag="wb")
        for isl in range(IT):
            sl = slice(isl * P, (isl + 1) * P)
            nc.tensor.transpose(ps_zT[:, sl], zbf[:, isl, :], idb)
        zT = hp.tile([NH, R], dt.bfloat16, tag="hT_s")
        nc.vector.tensor_copy(zT, ps_zT)
        # s2 = z @ (W_out a) comes straight from zT (host-folded wa2 in
        # aout_sb) so the w2/Wb2 chain runs in parallel with h2T.
        ps_s2 = pa.tile([2, R], dt.float32, tag="sall")
        nc.tensor.matmul(ps_s2, aout_sb, zT, start=True, stop=True)
        s2row = sp.tile([2, R], dt.bfloat16, tag="srow_s")
        nc.any.tensor_copy(s2row, ps_s2)
        ps_h2T = pa.tile([NO, R], dt.float32, tag="srow")
        nc.tensor.matmul(ps_h2T, wout_sb, zT, start=True, stop=True)
        h2T = hp.tile([NO, R], dt.bfloat16, tag="h2T_s")
        nc.any.tensor_copy(h2T, ps_h2T)
        w2row = sp.tile([1, R], dt.bfloat16, tag="wrow")
        nc.scalar.activation(w2row, s2row[0:1, :], Act.Exp, scale=0.8)
        ps_wb2 = pa.tile([P, R], dt.float32, tag="wb")
        nc.tensor.matmul(ps_wb2, ones1, w2row, start=True, stop=True)
        Wb2 = wp.tile([P, R], dt.bfloat16, tag="Wb")
        nc.vector.tensor_copy(Wb2, ps_wb2)
        ps_h2n = pa.tile([P, IT, NO + 2], dt.bfloat16, tag="hn", bufs=2)
        for isl in range(IT):
            sl = slice(isl * P, (isl + 1) * P)
            nc.tensor.transpose(ps_h2n[:, isl, 0:NO], h2T[:, sl],
                                idb[0:NO, 0:NO])
            nc.tensor.transpose(ps_h2n[:, isl, NO:NO + 2], s2row[:, sl],
                                idb[0:2, 0:2])
        v2col = sp.tile([P, IT], dt.float32, tag="vcol")
        nc.scalar.activation(v2col, ps_h2n[:, :, NO + 1], Act.Exp,
                             scale=0.2)
        pay2 = pp2.tile([P, IT, NO + 2], dt.bfloat16, tag="pay")
        for isl in range(IT):
            nc.vector.tensor_scalar(pay2[:, isl, 0:NO],
                                    ps_h2n[:, isl, 0:NO],
                                    v2col[:, isl:isl + 1], None, Alu.mult)
        nc.vector.tensor_copy(pay2[:, :, NO], v2col)
        nc.scalar.activation(pay2[:, :, NO + 1], ps_h2n[:, :, NO + 1],
                             Act.Exp, scale=0.8)
        nc.sync.dma_start(out=ag2_in.rearrange("(isl p) c -> p isl c",
                                                    p=P),
                          in_=pay2)
        nc.gpsimd.collective_compute(
            "AllGather", Alu.bypass,
            ins=[ag2_in.opt()], outs=[ag2_out.opt()],
            replica_groups=[list(range(NCORES))])

        # ================= phase E: attention-2 + softmax ===============
        src2 = ag2_out.rearrange("(jb p) c -> p jb c", p=P)
        ps_t2 = attention(-1, Wb2, src2, NO + 1, "tr")
        rc4 = sp.tile([P, IT], dt.float32, tag="rc4")
        nc.vector.reciprocal(rc4, ps_t2[:, :, NO])
        ue = cp.tile([P, IT, NO], dt.float32, tag="ue")
        for isl in range(IT):
            nc.scalar.activation(ue[:, isl, :], ps_t2[:, isl, 0:NO],
                                 Act.Copy, scale=rc4[:, isl:isl + 1])
        e2 = cp.tile([P, IT, NO], dt.float32, tag="e2")
        nc.scalar.activation(e2, ue, Act.Exp)
        t1 = cp.tile([P, IT, NO], dt.float32, tag="t1")
        nc.vector.tensor_scalar(t1, e2, 1.0, -1.0, Alu.min, Alu.add)
        el = cp.tile([P, IT, NO], dt.float32, tag="el")
        nc.vector.scalar_tensor_tensor(el, ue, 0.0, t1, Alu.max, Alu.add)
        # softmax without max-subtraction: elu output is O(1), exp is safe
        ex = cp.tile([P, IT, NO], dt.float32, tag="ex")
        nc.scalar.activation(ex, el, Act.Exp)
        sm = sp.tile([P, IT, 1], dt.float32, tag="sm")
        nc.vector.tensor_reduce(sm, ex, AX.X, Alu.add)
        rc2 = sp.tile([P, IT], dt.float32, tag="rc2")
        nc.vector.reciprocal(rc2, sm[:, :, 0])
        oo = cp.tile([P, IT, NO], dt.float32, tag="oo")
        for isl in range(IT):
            nc.scalar.activation(oo[:, isl, :], ex[:, isl, :],
                                 Act.Copy, scale=rc2[:, isl:isl + 1])
        nc.sync.dma_start(
            out=out_d.ap().rearrange("(isl p) c -> p isl c", p=P),
            in_=oo)


def _prep_inputs(x, adj, Ws, As, W_out, a_out):
    x32 = np.asarray(x, np.float32)
    adj_np = np.asarray(adj)
    mask_full = adj_np > 0
    Ws32 = np.asarray(Ws, np.float32)              # [8, 512, 64]
    wkt = np.zeros((KH // 2, 4, P, P), np.float32)
    for pr in range(KH // 2):
        pairw = np.concatenate([Ws32[2 * pr], Ws32[2 * pr + 1]],
                               axis=1)              # [512, 128]
        wkt[pr] = pairw.reshape(4, P, P)
    wkt = np.ascontiguousarray(wkt.transpose(2, 0, 1, 3)).astype(bf16)
    av = np.asarray(As, np.float64)[:, :, 0]        # [8, 128]
    Ws64 = np.asarray(Ws, np.float64)
    wa = np.zeros((KH, 2, F), np.float64)           # (head, a1/a2, feat)
    for k in range(KH):
        wa[k, 0] = Ws64[k] @ av[k, :NH]
        wa[k, 1] = Ws64[k] @ av[k, NH:]
    waall = np.zeros((P, 4, 2 * KH), np.float32)
    for k in range(KH):
        for c in range(2):
            waall[:, :, 2 * k + c] = wa[k, c].reshape(4, P).T
    waall = np.ascontiguousarray(waall).astype(bf16)
    wab = np.zeros((P, KH // 2, 4, 33), np.float32)
    for pr in range(KH // 2):
        wab[:, pr, :, 0] = wa[2 * pr, 0].reshape(4, P).T
        wab[:, pr, :, 32] = wa[2 * pr + 1, 0].reshape(4, P).T
    wab = np.ascontiguousarray(wab).astype(bf16)
    wout = np.asarray(W_out, np.float32).astype(bf16)
    Wo64 = np.asarray(W_out, np.float64)
    ao = np.asarray(a_out, np.float64)[:, 0]
    aout = np.ascontiguousarray(
        np.stack([Wo64 @ ao[:NO], Wo64 @ ao[NO:]], axis=-1)
        .astype(np.float32)).astype(bf16)
    sel = np.zeros((P, 4, NH + 1), np.float32)     # (unused placeholder)
    # full x^T (replicated) + head-0 [W_0 | W_0 a2_0] for the local
    # head-0 payload computed during the barrier window
    xTf = np.ascontiguousarray(
        x32.T.reshape(4, P, N).transpose(1, 0, 2)).astype(bf16)
    wk0a = np.zeros((P, 4, NH + 1), np.float32)
    for fb in range(4):
        wk0a[:, fb, 0:NH] = Ws32[0][fb * P:(fb + 1) * P, :]
        wk0a[:, fb, NH] = wa[0, 1].reshape(4, P)[fb]
    wk0a = np.ascontiguousarray(wk0a).astype(bf16)

    in_maps = []
    for d in range(NCORES):
        rows = slice(R * d, R * (d + 1))
        xT = np.ascontiguousarray(
            x32[rows].T.reshape(4, P, R).transpose(1, 0, 2)).astype(bf16)
        maskT = np.ascontiguousarray(
            mask_full[rows].T.astype(bf16).reshape(NB, P, R)
            .transpose(1, 0, 2))
        in_maps.append({
            "xT": xT, "maskT": maskT, "wkt": wkt, "waall": waall,
            "wab": wab,
            "wout": wout, "aout": aout, "sel": sel,
            "xTf": xTf, "wk0a": wk0a,
        })
    return in_maps


def kernel(x, adj, Ws, As, W_out, a_out, trace=False):
    if "nc" not in _CACHE:
        _CACHE["nc"] = _build()
    nc = _CACHE["nc"]
    in_maps = _prep_inputs(x, adj, Ws, As, W_out, a_out)
    res = run_bass_kernel_spmd(nc, in_maps, list(range(NCORES)), trace=trace)
    out = np.concatenate([res.results[d]["out"] for d in range(NCORES)],
                         axis=0).astype(np.float32)
    if trace:
        kernel.last_exec_time_ns = res.exec_time_ns
    return out



# revision 52
# speedup vs baseline: 1.1520x; 1.1520x over previous
"""GAT (2-layer, 8-head) Bass kernel for 8 Trainium2 NeuronCores.

Strategy (row-sharded attention):
  - Core d owns 512 rows (nodes) i in [512d, 512d+512).
  - Layer-1: each core computes h_k = x_d @ W_k for its rows (all 8 heads),
    plus score vectors; one AllGather shares [v_j*h_j | v_j | z_j] for all
    nodes; each core then computes its row-block of masked attention and
    h'_k via PE matmuls, using the separable-exponential decomposition
        exp(leakyrelu(s1_i + s2_j)) = u_i * v_j * max(w_i * z_j, 1)
    with u = exp(.2 s1) (cancels in softmax), v = exp(.2 s2),
    w = exp(.8 s1), z = exp(.8 s2).
    The unnormalized masked score matrix (transposed, [j,i]-layout) is
        U[j,i] = max(Wb[j,i] * z_j, 1) * mask[j,i]      (v folded into h)
    built with one dual-scalar TENSOR_SCALAR + one TENSOR_TENSOR per tile,
    then contracted on the PE against [v*h | v] to get numerator and
    denominator in one PSUM accumulation.
  - Layer-2: same pipeline once more on z = mean_k elu(h'_k).

The mask (adj > 0) is pre-transposed and bf16-encoded on the host; adj is
never touched on-device in int32 form.
"""

import numpy as np
import ml_dtypes

import concourse.bass as bass
import concourse.bacc as bacc
import concourse.tile as tile
import concourse.mybir as mybir
from concourse.bass_utils import run_bass_kernel_spmd
from concourse.masks import make_identity

dt = mybir.dt
Alu = mybir.AluOpType
Act = mybir.ActivationFunctionType
AX = mybir.AxisListType

NCORES = 8
N, F, NH, KH, NO = 4096, 512, 64, 8, 56
P = 128
R = N // NCORES          # rows per core = 512
IT = R // P              # i-tiles per core = 4
NB = N // P              # j-blocks = 32
GRP = 4                  # j-blocks per TT group
NG = NB // GRP           # groups = 8
bf16 = ml_dtypes.bfloat16

_CACHE: dict = {}


def _build():
    nc = bacc.Bacc("TRN2", target_bir_lowering=False, debug=False,
                   num_devices=NCORES)

    # ---- I/O -----------------------------------------------------------
    xT_d = nc.dram_tensor("xT", [P, 4, R], dt.bfloat16, kind="ExternalInput")
    maskT_d = nc.dram_tensor("maskT", [P, NB, R], dt.bfloat16,
                             kind="ExternalInput")
    wkt_d = nc.dram_tensor("wkt", [P, KH // 2, 4, P], dt.bfloat16,
                           kind="ExternalInput")
    waall_d = nc.dram_tensor("waall", [P, 4, 2 * KH], dt.bfloat16,
                             kind="ExternalInput")
    wab_d = nc.dram_tensor("wab", [P, KH // 2, 4, 33], dt.bfloat16,
                           kind="ExternalInput")
    wout_d = nc.dram_tensor("wout", [NH, NO], dt.bfloat16,
                            kind="ExternalInput")
    aout_d = nc.dram_tensor("aout", [NH, 2], dt.bfloat16,
                            kind="ExternalInput")
    sel_d = nc.dram_tensor("sel", [P, 4, NH + 1], dt.float32,
                           kind="ExternalInput")
    xTf_d = nc.dram_tensor("xTf", [P, 4, N], dt.bfloat16,
                           kind="ExternalInput")
    wk0a_d = nc.dram_tensor("wk0a", [P, 4, NH + 1], dt.bfloat16,
                            kind="ExternalInput")
    out_d = nc.dram_tensor("out", [R, NO], dt.float32, kind="ExternalOutput")

    with tile.TileContext(nc) as tc:
        _emit(nc, tc, xT_d, maskT_d, wkt_d, waall_d, wab_d, wout_d, aout_d,
              sel_d, xTf_d, wk0a_d, out_d)

    nc.compile()
    return nc


def _emit(nc, tc, xT_d, maskT_d, wkt_d, waall_d, wab_d, wout_d, aout_d,
              sel_d, xTf_d, wk0a_d, out_d):
    from contextlib import ExitStack
    ctx = ExitStack()
    with ctx:
        const = ctx.enter_context(tc.tile_pool(name="const", bufs=1))
        dram = ctx.enter_context(tc.tile_pool(name="dram", bufs=1,
                                              space="DRAM"))
        # PSUM pools: phase-A style (bufs=1) and attention (bufs=2)
        pa = ctx.enter_context(tc.tile_pool(name="pa", bufs=1, space="PSUM"))
        pb = ctx.enter_context(tc.tile_pool(name="pb", bufs=2, space="PSUM"))
        pt = ctx.enter_context(tc.tile_pool(name="pt", bufs=1, space="PSUM"))
        # SBUF working pools
        sp = ctx.enter_context(tc.tile_pool(name="sp", bufs=3))
        wp = ctx.enter_context(tc.tile_pool(name="wp", bufs=8))
        hp = ctx.enter_context(tc.tile_pool(name="hp", bufs=2))
        pp2 = ctx.enter_context(tc.tile_pool(name="pp2", bufs=4))
        tp = ctx.enter_context(tc.tile_pool(name="tp", bufs=3))
        up = ctx.enter_context(tc.tile_pool(name="up", bufs=3))
        bp = ctx.enter_context(tc.tile_pool(name="bp", bufs=3))
        cp = ctx.enter_context(tc.tile_pool(name="cp", bufs=2))
        zp = ctx.enter_context(tc.tile_pool(name="zp", bufs=1))

        # ---- resident loads (all partition-major, contiguous) ---------
        xT_sb = const.tile([P, 4, R], dt.bfloat16)
        nc.sync.dma_start(out=xT_sb, in_=xT_d.ap())
        wkp_sb = const.tile([P, KH // 2, 4, P], dt.bfloat16)
        nc.sync.dma_start(out=wkp_sb, in_=wkt_d.ap())
        waall_sb = const.tile([P, 4, 2 * KH], dt.bfloat16)
        nc.sync.dma_start(out=waall_sb, in_=waall_d.ap())
        wab_sb = const.tile([P, KH // 2, 4, 33], dt.bfloat16)
        nc.sync.dma_start(out=wab_sb, in_=wab_d.ap())
        wout_sb = const.tile([NH, NO], dt.bfloat16)
        nc.sync.dma_start(out=wout_sb, in_=wout_d.ap())
        sel_sb = const.tile([P, 4, NH + 1], dt.float32)
        nc.sync.dma_start(out=sel_sb, in_=sel_d.ap())
        aout_sb = const.tile([NH, 2], dt.bfloat16)
        nc.sync.dma_start(out=aout_sb, in_=aout_d.ap())
        wk0a_sb = const.tile([P, 4, NH + 1], dt.bfloat16)
        nc.gpsimd.dma_start(out=wk0a_sb, in_=wk0a_d.ap())
        xTf_sb = const.tile([P, 4, N], dt.bfloat16)
        mask_sb = const.tile([P, NB, R], dt.bfloat16)
        for mh in range(2):
            lo, hi = mh * (NB // 2), (mh + 1) * (NB // 2)
            meng = nc.scalar if mh == 0 else nc.sync
            meng.dma_start(out=mask_sb[:, lo:hi, :],
                           in_=maskT_d.ap()[:, lo:hi, :])
        # replicated full x^T last: it is only needed once phase A is done
        # (local head-0 build), so it must not delay xT/wkt/mask
        nc.sync.dma_start(out=xTf_sb, in_=xTf_d.ap())

        idb = const.tile([P, P], dt.bfloat16)
        make_identity(nc, idb)
        idf = const.tile([P, P], dt.float32)
        make_identity(nc, idf)
        neg1 = const.tile([P, 1], dt.float32)
        nc.vector.memset(neg1, -1.0)
        log8 = const.tile([P, 1], dt.float32)
        nc.vector.memset(log8, -2.0794415416798357)  # ln(1/8)
        ones1 = const.tile([1, P], dt.bfloat16)
        nc.vector.memset(ones1, 1.0)
        zero1 = const.tile([P, 1], dt.bfloat16)
        nc.vector.memset(zero1, 0.0)

        # persistent accumulators
        u_all = zp.tile([P, KH, IT, NH], dt.float32)   # h'_k per head (divided)
        zacc = zp.tile([P, IT, NH], dt.float32)        # mean elu
        zbf = zp.tile([P, IT, NH], dt.bfloat16)

        # head 0 is computed locally from the replicated full x during the
        # first-collective barrier window; only heads 1-7 are gathered.
        AG_SPLIT = [1, 2, 2, 2]                    # heads per gather wave
        ag_ins, ag_outs = [], []
        for nk in AG_SPLIT:
            ag_ins.append(dram.tile([R, nk, NH + 2], dt.bfloat16,
                                    name=f"agi{len(ag_ins)}"))
            ag_outs.append(dram.tile([N, nk, NH + 2], dt.bfloat16,
                                     addr_space="Shared",
                                     name=f"ago{len(ag_outs)}"))

        def head_wave(k):
            return (0, 0) if k == 1 else ((k - 2) // 2 + 1, (k - 2) % 2)
        ag2_in = dram.tile([R, NO + 2], dt.bfloat16)
        ag2_out = dram.tile([N, NO + 2], dt.bfloat16, addr_space="Shared")



        # ================= phase A: batched h + payload =================
        # h is computed directly in natural [i, hid] layout (x^T slices
        # stationary, W streamed), s-columns for all 8 heads come from one
        # host-precomputed (W @ a) stationary, and the w-rows from a tiny
        # dedicated matmul — no transposes, no cross-engine chains.
        ps_sall = pa.tile([P, IT, 2 * KH], dt.float32, tag="sall")
        scl_sb = sp.tile([P, IT, 2 * KH], dt.bfloat16, tag="scl", bufs=1)

        def hn_batch(half):
            prs = [2 * half, 2 * half + 1]
            ps_hns = {pr: pa.tile([P, IT, P], dt.float32, tag="hn", bufs=2,
                                  name=f"ps_hn{pr}")
                      for pr in prs}
            for isl in range(IT):
                sl = slice(isl * P, (isl + 1) * P)
                for fb in range(4):
                    lhsT = xT_sb[:, fb, sl]
                    for pr in prs:
                        nc.tensor.matmul(ps_hns[pr][:, isl, :], lhsT,
                                         wkp_sb[:, pr, fb, :],
                                         start=(fb == 0), stop=(fb == 3))
                    if half == 0:
                        nc.tensor.matmul(ps_sall[:, isl, :], lhsT,
                                         waall_sb[:, fb, :],
                                         start=(fb == 0), stop=(fb == 3))
                if half == 0:
                    nc.any.tensor_copy(scl_sb[:, isl, :], ps_sall[:, isl, :])
            return ps_hns

        def pair_payload(pr, ps_hn):
            Wbs = []
            ps_s = pa.tile([33, R], dt.float32, tag="srow")
            for fb in range(4):
                nc.tensor.matmul(ps_s, wab_sb[:, pr, fb, :],
                                 xT_sb[:, fb, :],
                                 start=(fb == 0), stop=(fb == 3))
            for hh in range(2):
                k = 2 * pr + hh
                wrow = sp.tile([1, R], dt.bfloat16, tag="wrow")
                nc.scalar.activation(wrow, ps_s[32 * hh:32 * hh + 1, :],
                                     Act.Exp, scale=0.8)
                ps_wb = pa.tile([P, R], dt.float32, tag="wb")
                nc.tensor.matmul(ps_wb, ones1, wrow, start=True, stop=True)
                Wb = wp.tile([P, R], dt.bfloat16, tag="Wb")
                nc.vector.tensor_copy(Wb, ps_wb)
                Wbs.append(Wb)
                if k == 0:
                    continue        # head 0's j-side is computed locally
                vcol = sp.tile([P, IT], dt.float32, tag="vcol")
                nc.scalar.activation(vcol, scl_sb[:, :, 2 * k + 1], Act.Exp,
                                     scale=0.2)
                pay = pp2.tile([P, IT, NH + 2], dt.bfloat16, tag="pay")
                for isl in range(IT):
                    nc.scalar.activation(
                        pay[:, isl, 0:NH],
                        ps_hn[:, isl, hh * NH:(hh + 1) * NH],
                        Act.Copy, scale=vcol[:, isl:isl + 1])
                nc.vector.tensor_copy(pay[:, :, NH], vcol)
                nc.scalar.activation(pay[:, :, NH + 1], scl_sb[:, :, 2 * k + 1],
                                     Act.Exp, scale=0.8)
                wave, kk = head_wave(k)
                nc.sync.dma_start(
                    out=ag_ins[wave][:, kk, :].rearrange(
                        "(isl p) c -> p isl c", p=P),
                    in_=pay)
            return Wbs

        Wb_k = []
        for half in range(2):
            ps_hns = hn_batch(half)
            for pr in (2 * half, 2 * half + 1):
                Wb_k += pair_payload(pr, ps_hns[pr])
                nc.gpsimd.collective_compute(
                    "AllGather", Alu.bypass,
                    ins=[ag_ins[pr].opt()], outs=[ag_outs[pr].opt()],
                    replica_groups=[list(range(NCORES))])

        # ---- local full-graph payload for head 0 -----------------------
        # Runs in the otherwise-idle window while the runtime's
        # first-collective barrier + wave-0 AllGather complete: h, v, z
        # for ALL 4096 nodes of head 0 from the replicated x^T.
        hsb0 = const.tile([P, NB, NH + 2], dt.bfloat16)
        for c in range(NG):
            ps_a0 = pt.tile([P, 4, NH + 1], dt.float32, tag="tr")
            for jj in range(4):
                jb = 4 * c + jj
                for fb in range(4):
                    nc.tensor.matmul(ps_a0[:, jj, :],
                                     xTf_sb[:, fb, jb * P:(jb + 1) * P],
                                     wk0a_sb[:, fb, :],
                                     start=(fb == 0), stop=(fb == 3))
            vq = sp.tile([P, 4], dt.float32, tag="vq")
            nc.scalar.activation(vq, ps_a0[:, :, NH], Act.Exp, scale=0.2)
            nc.scalar.activation(hsb0[:, 4 * c:4 * c + 4, NH + 1],
                                 ps_a0[:, :, NH], Act.Exp, scale=0.8)
            nc.vector.tensor_copy(hsb0[:, 4 * c:4 * c + 4, NH], vq)
            for jj in range(4):
                nc.vector.tensor_scalar(hsb0[:, 4 * c + jj, 0:NH],
                                        ps_a0[:, jj, 0:NH],
                                        vq[:, jj:jj + 1], None, Alu.mult)

        # ================= phase B: per-head attention ==================
        def attention(k, Wb, hsb_src, ncols, ps_tag, hsb_local=None):
            """hsb_src: dram AP [P, NB, ncols+1]-gatherable; returns psum_t
            [P, IT, ncols] (last col = denominator)."""
            if hsb_local is not None:
                hsb = hsb_local
            else:
                hsb = bp.tile([P, NB, ncols + 1], dt.bfloat16, tag="hsb")
                nc.sync.dma_start(out=hsb, in_=hsb_src)
            zf = bp.tile([P, NB], dt.float32, tag="zf")
            nc.vector.tensor_copy(zf, hsb[:, :, ncols])
            ps_nm = pb.tile([ncols, R], dt.float32, tag="nm")
            # Per group of 4 j-blocks: the first NACT tiles are produced on
            # the ACT engine as T = relu(z*Wb - 1); their missing "+1" is
            # recovered by streaming the resident mask tile itself through
            # the PE into the same PSUM accumulation (numer += sum_j m*h).
            # The other tiles use the DVE dual-scalar max(z*Wb, 1) form.
            # One plain TENSOR_TENSOR applies the mask to all 4 tiles.
            NACT = 2
            for g in range(NG):
                Tg = tp.tile([P, GRP, R], dt.bfloat16, tag="T")
                for q in range(NACT):
                    jb = g * GRP + q
                    nc.scalar.activation(Tg[:, q, :], Wb, Act.Relu,
                                         bias=neg1[:, 0:1],
                                         scale=zf[:, jb:jb + 1])
                for q in range(NACT, GRP):
                    jb = g * GRP + q
                    nc.vector.tensor_scalar(Tg[:, q, :], Wb,
                                            zf[:, jb:jb + 1], 1.0,
                                            Alu.mult, Alu.max)
                Ug = up.tile([P, GRP, R], dt.bfloat16, tag="U")
                nc.vector.tensor_tensor(Ug, Tg,
                                        mask_sb[:, g * GRP:(g + 1) * GRP, :],
                                        Alu.mult)
                for q in range(GRP):
                    jb = g * GRP + q
                    nc.tensor.matmul(ps_nm, hsb[:, jb, 0:ncols], Ug[:, q, :],
                                     start=(jb == 0), stop=(jb == NB - 1))
                    if q < NACT:
                        nc.tensor.matmul(ps_nm, hsb[:, jb, 0:ncols],
                                         mask_sb[:, jb, :],
                                         start=False, stop=False)
            nmf = bp.tile([ncols, R], dt.float32, tag="nmf")
            nc.any.tensor_copy(nmf, ps_nm)
            ps_t = pt.tile([P, IT, ncols], dt.float32, tag=ps_tag)
            for isl in range(IT):
                sl = slice(isl * P, (isl + 1) * P)
                nc.tensor.transpose(ps_t[:, isl, :], nmf[:, sl],
                                    idf[0:ncols, 0:ncols])
            return ps_t

        # phase B + incremental phase C: after each head's normalize, its
        # elu contribution is folded into zacc (init -1, += relu(u)/8 +
        # min(exp(u),1)/8 per head) so no serial elu/mean block remains
        # after the last head.
        nc.vector.memset(zacc, -1.0)
        for k in range(KH):
            if k == 0:
                ps_t = attention(0, Wb_k[0], None, NH + 1, "tr",
                                 hsb_local=hsb0)
            else:
                wave, kk = head_wave(k)
                src = ag_outs[wave][:, kk, :].rearrange(
                    "(jb p) c -> p jb c", p=P)
                ps_t = attention(k, Wb_k[k], src, NH + 1, "tr")
            for isl in range(IT):
                rc = sp.tile([P, 1], dt.float32, tag="rc")
                nc.vector.reciprocal(rc, ps_t[:, isl, NH:NH + 1])
                nc.scalar.activation(u_all[:, k, isl, :],
                                     ps_t[:, isl, 0:NH],
                                     Act.Copy, scale=rc[:, 0:1])
            uk = u_all[:, k, :, :]
            dp = cp.tile([P, IT, NH], dt.float32, tag="cdp")
            nc.scalar.activation(dp, uk, Act.Relu, scale=0.125)
            rn = cp.tile([P, IT, NH], dt.float32, tag="crn")
            nc.scalar.activation(rn, uk, Act.Relu, scale=-1.0)
            bq = cp.tile([P, IT, NH], dt.float32, tag="cbq")
            nc.scalar.activation(bq, rn, Act.Exp, bias=log8[:, 0:1],
                                 scale=-1.0)
            nc.gpsimd.tensor_tensor(zacc, zacc, dp, Alu.add)
            nc.gpsimd.tensor_tensor(zacc, zacc, bq, Alu.add)
        nc.vector.tensor_copy(zbf, zacc)

        # ================= phase D: layer-2 h2 + payload ================
        ps_zT = pa.tile([NH, R], dt.bfloat16, tag="wb")
        for isl in range(IT):
            sl = slice(isl * P, (isl + 1) * P)
            nc.tensor.transpose(ps_zT[:, sl], zbf[:, isl, :], idb)
        zT = hp.tile([NH, R], dt.bfloat16, tag="hT_s")
        nc.vector.tensor_copy(zT, ps_zT)
        # s2 = z @ (W_out a) comes straight from zT (host-folded wa2 in
        # aout_sb) so the w2/Wb2 chain runs in parallel with h2T.
        ps_s2 = pa.tile([2, R], dt.float32, tag="sall")
        nc.tensor.matmul(ps_s2, aout_sb, zT, start=True, stop=True)
        s2row = sp.tile([2, R], dt.bfloat16, tag="srow_s")
        nc.any.tensor_copy(s2row, ps_s2)
        ps_h2T = pa.tile([NO, R], dt.float32, tag="srow")
        nc.tensor.matmul(ps_h2T, wout_sb, zT, start=True, stop=True)
        h2T = hp.tile([NO, R], dt.bfloat16, tag="h2T_s")
        nc.any.tensor_copy(h2T, ps_h2T)
        w2row = sp.tile([1, R], dt.bfloat16, tag="wrow")
        nc.scalar.activation(w2row, s2row[0:1, :], Act.Exp, scale=0.8)
        ps_wb2 = pa.tile([P, R], dt.float32, tag="wb")
        nc.tensor.matmul(ps_wb2, ones1, w2row, start=True, stop=True)
        Wb2 = wp.tile([P, R], dt.bfloat16, tag="Wb")
        nc.vector.tensor_copy(Wb2, ps_wb2)
        ps_h2n = pa.tile([P, IT, NO + 2], dt.bfloat16, tag="hn", bufs=2)
        for isl in range(IT):
            sl = slice(isl * P, (isl + 1) * P)
            nc.tensor.transpose(ps_h2n[:, isl, 0:NO], h2T[:, sl],
                                idb[0:NO, 0:NO])
            nc.tensor.transpose(ps_h2n[:, isl, NO:NO + 2], s2row[:, sl],
                                idb[0:2, 0:2])
        v2col = sp.tile([P, IT], dt.float32, tag="vcol")
        nc.scalar.activation(v2col, ps_h2n[:, :, NO + 1], Act.Exp,
                             scale=0.2)
        pay2 = pp2.tile([P, IT, NO + 2], dt.bfloat16, tag="pay")
        for isl in range(IT):
            nc.vector.tensor_scalar(pay2[:, isl, 0:NO],
                                    ps_h2n[:, isl, 0:NO],
                                    v2col[:, isl:isl + 1], None, Alu.mult)
        nc.vector.tensor_copy(pay2[:, :, NO], v2col)
        nc.scalar.activation(pay2[:, :, NO + 1], ps_h2n[:, :, NO + 1],
                             Act.Exp, scale=0.8)
        nc.sync.dma_start(out=ag2_in.rearrange("(isl p) c -> p isl c",
                                                    p=P),
                          in_=pay2)
        nc.gpsimd.collective_compute(
            "AllGather", Alu.bypass,
            ins=[ag2_in.opt()], outs=[ag2_out.opt()],
            replica_groups=[list(range(NCORES))])

        # ================= phase E: attention-2 + softmax ===============
        src2 = ag2_out.rearrange("(jb p) c -> p jb c", p=P)
        ps_t2 = attention(-1, Wb2, src2, NO + 1, "tr")
        rc4 = sp.tile([P, IT], dt.float32, tag="rc4")
        nc.vector.reciprocal(rc4, ps_t2[:, :, NO])
        ue = cp.tile([P, IT, NO], dt.float32, tag="ue")
        for isl in range(IT):
            nc.scalar.activation(ue[:, isl, :], ps_t2[:, isl, 0:NO],
                                 Act.Copy, scale=rc4[:, isl:isl + 1])
        e2 = cp.tile([P, IT, NO], dt.float32, tag="e2")
        nc.scalar.activation(e2, ue, Act.Exp)
        t1 = cp.tile([P, IT, NO], dt.float32, tag="t1")
        nc.vector.tensor_scalar(t1, e2, 1.0, -1.0, Alu.min, Alu.add)
        el = cp.tile([P, IT, NO], dt.float32, tag="el")
        nc.vector.scalar_tensor_tensor(el, ue, 0.0, t1, Alu.max, Alu.add)
        # softmax without max-subtraction: elu output is O(1), exp is safe
        ex = cp.tile([P, IT, NO], dt.float32, tag="ex")
        nc.scalar.activation(ex, el, Act.Exp)
        sm = sp.tile([P, IT, 1], dt.float32, tag="sm")
        nc.vector.tensor_reduce(sm, ex, AX.X, Alu.add)
        rc2 = sp.tile([P, IT], dt.float32, tag="rc2")
        nc.vector.reciprocal(rc2, sm[:, :, 0])
        oo = cp.tile([P, IT, NO], dt.float32, tag="oo")
        for isl in range(IT):
            nc.scalar.activation(oo[:, isl, :], ex[:, isl, :],
                                 Act.Copy, scale=rc2[:, isl:isl + 1])
        nc.sync.dma_start(
            out=out_d.ap().rearrange("(isl p) c -> p isl c", p=P),
            in_=oo)


def _prep_inputs(x, adj, Ws, As, W_out, a_out):
    x32 = np.asarray(x, np.float32)
    adj_np = np.asarray(adj)
    mask_full = adj_np > 0
    Ws32 = np.asarray(Ws, np.float32)              # [8, 512, 64]
    wkt = np.zeros((KH // 2, 4, P, P), np.float32)
    for pr in range(KH // 2):
        pairw = np.concatenate([Ws32[2 * pr], Ws32[2 * pr + 1]],
                               axis=1)              # [512, 128]
        wkt[pr] = pairw.reshape(4, P, P)
    wkt = np.ascontiguousarray(wkt.transpose(2, 0, 1, 3)).astype(bf16)
    av = np.asarray(As, np.float64)[:, :, 0]        # [8, 128]
    Ws64 = np.asarray(Ws, np.float64)
    wa = np.zeros((KH, 2, F), np.float64)           # (head, a1/a2, feat)
    for k in range(KH):
        wa[k, 0] = Ws64[k] @ av[k, :NH]
        wa[k, 1] = Ws64[k] @ av[k, NH:]
    waall = np.zeros((P, 4, 2 * KH), np.float32)
    for k in range(KH):
        for c in range(2):
            waall[:, :, 2 * k + c] = wa[k, c].reshape(4, P).T
    waall = np.ascontiguousarray(waall).astype(bf16)
    wab = np.zeros((P, KH // 2, 4, 33), np.float32)
    for pr in range(KH // 2):
        wab[:, pr, :, 0] = wa[2 * pr, 0].reshape(4, P).T
        wab[:, pr, :, 32] = wa[2 * pr + 1, 0].reshape(4, P).T
    wab = np.ascontiguousarray(wab).astype(bf16)
    wout = np.asarray(W_out, np.float32).astype(bf16)
    Wo64 = np.asarray(W_out, np.float64)
    ao = np.asarray(a_out, np.float64)[:, 0]
    aout = np.ascontiguousarray(
        np.stack([Wo64 @ ao[:NO], Wo64 @ ao[NO:]], axis=-1)
        .astype(np.float32)).astype(bf16)
    sel = np.zeros((P, 4, NH + 1), np.float32)     # (unused placeholder)
    # full x^T (replicated) + head-0 [W_0 | W_0 a2_0] for the local
    # head-0 payload computed during the barrier window
    xTf = np.ascontiguousarray(
        x32.T.reshape(4, P, N).transpose(1, 0, 2)).astype(bf16)
    wk0a = np.zeros((P, 4, NH + 1), np.float32)
    for fb in range(4):
        wk0a[:, fb, 0:NH] = Ws32[0][fb * P:(fb + 1) * P, :]
        wk0a[:, fb, NH] = wa[0, 1].reshape(4, P)[fb]
    wk0a = np.ascontiguousarray(wk0a).astype(bf16)

    in_maps = []
    for d in range(NCORES):
        rows = slice(R * d, R * (d + 1))
        xT = np.ascontiguousarray(
            x32[rows].T.reshape(4, P, R).transpose(1, 0, 2)).astype(bf16)
        maskT = np.ascontiguousarray(
            mask_full[rows].T.astype(bf16).reshape(NB, P, R)
            .transpose(1, 0, 2))
        in_maps.append({
            "xT": xT, "maskT": maskT, "wkt": wkt, "waall": waall,
            "wab": wab,
            "wout": wout, "aout": aout, "sel": sel,
            "xTf": xTf, "wk0a": wk0a,
        })
    return in_maps


def kernel(x, adj, Ws, As, W_out, a_out, trace=False):
    if "nc" not in _CACHE:
        _CACHE["nc"] = _build()
    nc = _CACHE["nc"]
    in_maps = _prep_inputs(x, adj, Ws, As, W_out, a_out)
    res = run_bass_kernel_spmd(nc, in_maps, list(range(NCORES)), trace=trace)
    out = np.concatenate([res.results[d]["out"] for d in range(NCORES)],
                         axis=0).astype(np.float32)
    if trace:
        kernel.last_exec_time_ns = res.exec_time_ns
    return out

